# revision 1
# baseline (speedup 1.0000x reference)
"""CapsMaxPool Trainium2 kernel.

x: [B=64, H=64, W=64, C=32, A=8] fp32 capsules. For each 2x2 spatial window
and capsule c, pick the candidate position whose capsule vector has the
largest L2 norm (first-max-wins over the window in row-major (ph, pw) order)
and emit that 8-atom vector. Output: [B, 32, 32, 32, 8].

Strategy (per core; batch sharded 8 ways -> 8 examples/core):
  - Layout: spatial rows on SBUF partitions ((b, ho) flattened = 256 rows,
    2 row tiles), (capsule, atom) on the free dim. One loaded tile xq holds
    [128, ph=2, w=2*WCH, ca=256]; the four window candidates are strided
    views (ph, w-parity). Loads are 4 DMAs/group (4 KiB contiguous DRAM
    chunks per partition row) on the SP HWDGE ring, 5 tiles deep.
  - ScalarE squares the whole tile in one fp32 op (fp32 is required for
    bit-exact argmax: fp16/bf16 squares flip 346/2691 of the 2.1M argmax
    decisions; PE-matmul reduction in fp32r flips 194; all rejected).
  - Atom sums via a pairwise fp32 add tree on DVE (t4/t2/s). The engines
    that could offload it are dead ends, measured: GPSIMD shares its SBUF
    ports with DVE (any concurrent Pool op slows DVE dramatically), and
    PE identity-matmul accumulation costs 4 cycles/element for fp32.
  - Tournament select on an fp16 shadow of the tile: ScalarE casts xq to
    xq16 (one op), then DVE copy_predicated runs on int32-BITCAST fp16
    pairs, halving select element counts: one paired strict-greater mask
    covers both (pw) semifinals (overwrite even-w slabs from odd-w slabs
    where the mask fires), one final pass resolves ph0- vs ph1-winner.
    Strict-greater masks == first-max-wins == jnp.argmax tie-breaking,
    computed on exact fp32 norms.
  - The winner slab is stored as fp16 (halves store-side HBM traffic;
    output rel-L2 ~2e-4 is pure fp16 rounding; selection is exact). The
    host upcasts to fp32. Stores ride the SP ring; the ACT ring only
    carries ScalarE work (squares + cast), keeping both engine streams
    free of head-of-line blocking; cast+store for group g-1 are emitted
    after group g's front half (software pipelining of the emission
    order, which is each engine's execution order).

Engine balance per group (measured-model): DVE ~7.3us, ScalarE ~7.2us,
DMA ~6.6us -> ~112-115us/rep device time vs the 105us HBM floor
(33.55 MB read + 4.19 MB fp16 write per core at ~358 GB/s/core).
"""

import numpy as np

import concourse.bass as bass
import concourse.tile as tile
from concourse import mybir
from concourse.bass_utils import run_bass_kernel_spmd

B, H, W, C, A = 64, 64, 64, 32, 8
PH = PW = 2
NCORES = 8
BL = B // NCORES          # batches per core
Ho, Wo = H // PH, W // PW
CA = C * A                # 256
R = BL * Ho               # 256 partition rows per core ((b, ho) flattened)
NRT = R // 128            # row tiles

F32 = mybir.dt.float32
F16 = mybir.dt.float16
U8 = mybir.dt.uint8


def _split_multi_waits(nc):
    """Walrus on this toolchain encodes at most ONE sync wait per
    instruction; Tile attaches several. Hoist all-but-one wait into
    standalone InstEventSemaphore ops just before the instruction (same
    engine stream position => identical semantics)."""
    for fn in nc.m.functions:
        for bb in fn.blocks:
            new = []
            for ins in bb.instructions:
                si = ins.sync_info
                if si is not None and si.on_wait and len(si.on_wait) > 1:
                    waits = list(si.on_wait)
                    for j, w in enumerate(waits[:-1]):
                        new.append(mybir.InstEventSemaphore(
                            name=f"{ins.name}-hw{j}",
                            engine=ins.engine,
                            ins=[], outs=[],
                            sync_info=mybir.SyncInfo(on_wait=[w], on_update=[]),
                        ))
                    ins.sync_info = mybir.SyncInfo(
                        on_wait=[waits[-1]], on_update=list(si.on_update)
                    )
                new.append(ins)
            bb.instructions = new


def _bcast_atoms(ap, n=A):
    """View an AP with an extra step-0 trailing dim of size n."""
    return bass.AP(tensor=ap.tensor, offset=ap.offset, ap=list(ap.ap) + [[0, n]])


def _group_front(nc, big, small, xvc, r0, w0, cfg, psum=None, ident=None):
    """Loads + squares + tree + masks + in-place select for one group.
    Returns (xq, winner view) for the deferred cast+store."""
    WCH = cfg["wch"]
    W2 = 2 * WCH

    xq = big.tile([128, PH, W2, CA], F32, name="xq", bufs=cfg.get("load_bufs"))
    nload = cfg.get("load_split", 2)
    nwh = nload // PH  # w-chunks per ph row (1 for split2, 2 for split4)
    cs = W2 // nwh
    for i in range(nload):
        ph, wh = divmod(i, nwh)
        eng = nc.scalar if (cfg.get("load_rings") and i % 2) else nc.sync
        eng.dma_start(
            out=xq[:, ph, wh * cs:(wh + 1) * cs, :],
            in_=xvc[r0:r0 + 128, ph, 2 * w0 + wh * cs:2 * w0 + (wh + 1) * cs, :],
        )

    # squares, one dense op over the whole tile (fp32; float32r when the
    # PE consumes them in single-pass mode, which requires pre-rounding)
    sq_dt = mybir.dt.float32r if cfg.get("pe_norms") == "f32r" else F32
    sq = big.tile([128, PH, W2, CA], sq_dt, name="sq", bufs=cfg.get("sq_bufs"))
    nc.scalar.activation(sq, xq, mybir.ActivationFunctionType.Square)

    if cfg.get("pe_norms"):
        # Atom-sum on the (otherwise idle) tensor engine: 8 accumulating
        # identity matmuls, one per atom-strided view of sq. The contraction
        # over partitions with I passes each row through; PSUM accumulates
        # over the 8 views = sum over atoms. fp32r streams 1 elem/cycle
        # (fp32 is 4-pass).
        sqa = sq[:].rearrange("p ph w (c a) -> p ph w c a", a=A)
        psum_s = psum.tile([128, PH, W2, C], F32, name="psum_s")
        for i in range(A):
            nc.tensor.matmul(
                out=psum_s[:], lhsT=ident[:], rhs=sqa[:, :, :, :, i],
                start=(i == 0), stop=(i == A - 1),
            )
        s = small.tile([128, PH, W2, C], F32, name="s")
        nc.scalar.copy(s, psum_s)
    else:
        # pairwise atom-sum tree on DVE: [ph, w, c, 8] -> [ph, w, c]
        sqv = sq[:].rearrange(
            "p ph w (c a2 two) -> p ph w c a2 two", a2=A // 2, two=2
        )
        t4 = small.tile([128, PH, W2, C, A // 2], F32, name="t4")
        l1 = nc.gpsimd if cfg.get("gps_t4") else nc.vector
        l1.tensor_add(t4, sqv[:, :, :, :, :, 0], sqv[:, :, :, :, :, 1])
        t4v = t4[:].rearrange("p ph w c (b2 two) -> p ph w c b2 two", two=2)
        t2 = small.tile([128, PH, W2, C, A // 4], F32, name="t2")
        l2 = nc.gpsimd if cfg.get("gps_t2s") else nc.vector
        l2.tensor_add(t2, t4v[:, :, :, :, :, 0], t4v[:, :, :, :, :, 1])
        t2v = t2[:].rearrange("p ph w c (b1 two) -> p ph w c b1 two", two=2)
        s = small.tile([128, PH, W2, C], F32, name="s")
        l2.tensor_add(s, t2v[:, :, :, :, 0, 0], t2v[:, :, :, :, 0, 1])

    # tournament masks: m2[ph] = (s(pw=1) > s(pw=0)), r2[ph] = max over pw,
    # mf = (r2[ph=1] > r2[ph=0]); strict-greater == first-max-wins.
    sv = s[:].rearrange("p ph (wo pw) c -> p ph wo pw c", pw=PW)
    r2 = small.tile([128, PH, WCH, C], F32, name="r2")
    if cfg.get("gps_masks"):
        # Pool TensorTensor wants one dtype across operands; the norms are
        # non-negative fp32, whose int32 bit patterns order identically.
        I32 = mybir.dt.uint32
        cast = lambda ap: ap.bitcast(I32)
        me = nc.gpsimd
        m2 = small.tile([128, PH, WCH, C], I32, name="m2")
        me.tensor_tensor(m2, cast(sv[:, :, :, 1]), cast(sv[:, :, :, 0]),
                         mybir.AluOpType.is_gt)
        me.tensor_tensor(cast(r2[:]), cast(sv[:, :, :, 0]), cast(sv[:, :, :, 1]),
                         mybir.AluOpType.max)
        mf = small.tile([128, WCH, C], I32, name="mf")
        me.tensor_tensor(mf, cast(r2[:, 1]), cast(r2[:, 0]), mybir.AluOpType.is_gt)
    else:
        m2 = small.tile([128, PH, WCH, C], U8, name="m2")
        nc.vector.tensor_tensor(m2, sv[:, :, :, 1], sv[:, :, :, 0],
                                mybir.AluOpType.is_gt)
        nc.vector.tensor_max(r2, sv[:, :, :, 0], sv[:, :, :, 1])
        mf = small.tile([128, WCH, C], U8, name="mf")
        nc.vector.tensor_tensor(mf, r2[:, 1], r2[:, 0], mybir.AluOpType.is_gt)

    I32 = mybir.dt.int32
    pack = cfg.get("pack16", "none")
    xqv = xq[:].rearrange("p ph (wo pw) ca -> p ph wo pw ca", pw=PW)
    if pack == "full":
        # cast the whole tile to fp16 up front, then run both tournament
        # rounds on int32-packed fp16 pairs (halves cp element counts)
        xq16 = small.tile([128, PH, W2, CA], F16, name="xq16",
                          bufs=cfg.get("x16_bufs"))
        if cfg.get("cast_gps"):
            nc.gpsimd.tensor_copy(xq16, xq)
        else:
            nc.scalar.copy(xq16, xq)
        x16 = xq16[:].rearrange("p ph (wo pw) ca -> p ph wo pw ca", pw=PW)
        pk4 = lambda ap: ap.bitcast(I32).rearrange(
            "p ph wo (c k) -> p ph wo c k", k=A // 2
        )
        pk3 = lambda ap: ap.bitcast(I32).rearrange(
            "p wo (c k) -> p wo c k", k=A // 2
        )
        nc.vector.copy_predicated(
            pk4(x16[:, :, :, 0]), _bcast_atoms(m2[:], A // 2),
            pk4(x16[:, :, :, 1]),
        )
        nc.vector.copy_predicated(
            pk3(x16[:, 0, :, 0]), _bcast_atoms(mf[:], A // 2),
            pk3(x16[:, 1, :, 0]),
        )
        return x16[:, 0, :, 0]

    # in-place f32 semifinal inside xq (even-w slabs hold the winners)
    dst_semi = xqv[:, :, :, 0].rearrange("p ph wo (c a) -> p ph wo c a", a=A)
    src_semi = xqv[:, :, :, 1].rearrange("p ph wo (c a) -> p ph wo c a", a=A)
    nc.vector.copy_predicated(dst_semi, _bcast_atoms(m2[:]), src_semi)
    if pack == "half":
        # defer the final to the back phase, on fp16-packed data
        return (xqv[:, :, :, 0], mf)
    dst_fin = xqv[:, 0, :, 0].rearrange("p wo (c a) -> p wo c a", a=A)
    src_fin = xqv[:, 1, :, 0].rearrange("p wo (c a) -> p wo c a", a=A)
    nc.vector.copy_predicated(dst_fin, _bcast_atoms(mf[:]), src_fin)
    return xqv[:, 0, :, 0]


def _group_back(nc, small, ov, r0, w0, winner, cfg):
    """Cast + store for one group (emitted with a 1-group lag so the ACT /
    SP streams never head-of-line-block the next group's squares/loads)."""
    WCH = cfg["wch"]
    oe = nc.scalar if cfg.get("out_act_ring") else nc.sync
    pack = cfg.get("pack16", "none")
    if cfg.get("out_f32"):
        oe.dma_start(out=ov[r0:r0 + 128, w0:w0 + WCH, :], in_=winner)
        return
    if pack == "full":
        oe.dma_start(out=ov[r0:r0 + 128, w0:w0 + WCH, :], in_=winner)
        return
    if pack == "half":
        semi, mf = winner
        o16 = small.tile([128, PH, WCH, CA], F16, name="o16",
                         bufs=cfg.get("out_bufs"))
        nc.scalar.copy(o16, semi)
        I32 = mybir.dt.int32
        pk = lambda ap: ap.bitcast(I32).rearrange(
            "p w (c k) -> p w c k", k=A // 2
        )
        nc.vector.copy_predicated(
            pk(o16[:, 0]), _bcast_atoms(mf[:], A // 2), pk(o16[:, 1])
        )
        oe.dma_start(out=ov[r0:r0 + 128, w0:w0 + WCH, :], in_=o16[:, 0])
        return
    o16 = small.tile([128, WCH, CA], F16, name="o16", bufs=cfg.get("out_bufs"))
    nc.scalar.copy(o16, winner)
    oe.dma_start(out=ov[r0:r0 + 128, w0:w0 + WCH, :], in_=o16)


DEFAULT_CFG = dict(
    wch=4, load_split=4, load_bufs=5, sq_bufs=2, out_bufs=2, bufs=4,
    pack16="full", out_act_ring=False, out_f32=False,
    gps_t4=False, gps_t2s=False, gps_masks=False,
)


def _build_bass(reps: int = 1, **overrides):
    """reps>1 repeats the whole per-core computation inside one NEFF —
    used by the timing harness to separate device time from launch/upload
    overhead ((T_reps - T_1) / (reps - 1))."""
    cfg = {**DEFAULT_CFG, **overrides}
    WCH = cfg["wch"]
    NWCH = Wo // WCH
    nc = bass.Bass()
    x = nc.dram_tensor("x", [BL, H, W, C, A], F32, kind="ExternalInput")
    ident_dt = mybir.dt.float32r if cfg.get("pe_norms") == "f32r" else F32
    ident_d = (
        nc.dram_tensor("ident", [128, 128], ident_dt, kind="ExternalInput")
        if cfg.get("pe_norms")
        else None
    )
    odt = F32 if cfg.get("out_f32") else F16
    out = nc.dram_tensor("out", [BL, Ho, Wo, C, A], odt, kind="ExternalOutput")

    # contiguous-load view: [(b ho), ph, w, ca] with w the full-res column;
    # (b, ho) merges because the b stride (H*W*C*A) is 32x the ho stride.
    xvc = x.rearrange("b (ho ph) w c a -> (b ho) ph w (c a)", ph=PH)
    ov = out.rearrange("b ho wo c a -> (b ho) wo (c a)")  # [256, 32, 256]

    with tile.TileContext(nc) as tc:
        with (
            tc.tile_pool(name="const", bufs=1) as const,
            tc.tile_pool(name="big", bufs=cfg["bufs"]) as big,
            tc.tile_pool(name="small", bufs=cfg["bufs"]) as small,
            tc.tile_pool(
                name="psum", bufs=cfg.get("psum_bufs", 2),
                space=bass.MemorySpace.PSUM,
            ) as psum,
        ):
            ident = None
            if cfg.get("pe_norms"):
                ident = const.tile([128, 128], ident_dt, name="ident")
                nc.sync.dma_start(out=ident, in_=ident_d[:, :])
            lag = cfg.get("lag", 1)
            pending = []  # (r0, w0, winner) tuples awaiting cast+store
            for _rep in range(reps):
                for rt in range(NRT):
                    for wq in range(NWCH):
                        r0, w0 = rt * 128, wq * WCH
                        winner = _group_front(
                            nc, big, small, xvc, r0, w0, cfg,
                            psum=psum, ident=ident,
                        )
                        pending.append((r0, w0, winner))
                        if len(pending) > lag:
                            _group_back(nc, small, ov, *pending.pop(0), cfg)
            for p in pending:
                _group_back(nc, small, ov, *p, cfg)
    _split_multi_waits(nc)
    return nc


def aux_inputs(nc) -> dict:
    """Extra constant inputs the module expects (beyond x), keyed by name."""
    names = set()
    for alloc in nc.m.functions[0].allocations:
        if isinstance(alloc, mybir.MemoryLocationSet) and alloc.kind == "ExternalInput":
            names.add(alloc.memorylocations[0].name)
    aux = {}
    if "ident" in names:
        aux["ident"] = np.eye(128, dtype=np.float32)
    return aux


_NC_CACHE = None


def kernel(x: np.ndarray) -> np.ndarray:
    global _NC_CACHE
    assert x.shape == (B, H, W, C, A) and x.dtype == np.float32
    if _NC_CACHE is None:
        _NC_CACHE = _build_bass()
    nc = _NC_CACHE

    shards = [
        np.ascontiguousarray(x[i * BL : (i + 1) * BL]) for i in range(NCORES)
    ]
    aux = aux_inputs(nc)
    in_maps = [{"x": s, **aux} for s in shards]
    res = run_bass_kernel_spmd(nc, in_maps, list(range(NCORES)))
    full = np.concatenate([r["out"] for r in res.results], axis=0)
    return np.ascontiguousarray(full.astype(np.float32))

